# revision 24
# baseline (speedup 1.0000x reference)
"""Trainium2 Bass kernel for the AttentionLoop module.

Reference computation (S=2048, B=32, D=1024, E=1024):
    h = tanh(einsum('sbd,ed->sbe', dec + enc, W_fc))
    scores = einsum('sbe,e->bs', h, score_w[:,0])
    attn = softmax(scores, axis=1)          # over seq
    out = einsum('bs,sbd->bd', attn, enc)   # (B, D)

Strategy: data-parallel over batch across 8 NeuronCores (4 batches/core),
core-local, no collectives.

Per-core kernel (v3). Heavy matmuls in float32r (1 cyc/row, full fp32 bits,
~1.5e-4 matmul rel err). The main matmul makes h (s, e)-oriented:
stationary = encT s-chunk (K=d, M=s), moving = W_T (K=d, N=e):
  - dc-outer / e-half-inner matmul order so each LDWEIGHTS (fp32r weight
    load is as long as one 512-col matmul) amortizes over two matmuls,
  - decoder bias decW[b, e] (varies along e) is a VectorE tensor_add on the
    PSUM tile against a pre-broadcast decw tile; decW is computed on-device
    by one M=4 matmul chain and replicated across partitions by GpSimd
    partition_broadcast,
  - scores[s] = sum_e h[s,e] sw[e] is one fused VectorE scalar_tensor_tensor
    (mult + accum_out) per s-chunk, landing directly as a column,
  - exp runs per column as soon as its scores are ready, so the pass-2
    weighted-sum matmuls (p column stationary, natural-enc moving)
    interleave with pass-1 and keep the TensorE dense.
W and encT DMAs are split per d-chunk so the first matmul starts ~3us in.
Softmax skips max-subtraction (scores are O(1); exp is safe in fp32).
l = sum(p) via DVE reduce + ones-matmul partition sum; 1/l is folded into
the final PSUM evacuation.
"""

import numpy as np

S, B, D, E = 2048, 32, 1024, 1024
NCORES = 8
BLOC = B // NCORES          # 4 batches per core
P = 128                     # partitions
DC = D // P                 # 8 d-chunks
SB = 512                    # moving free dim (PSUM bank)
NSBLK = S // SB             # 4 s-blocks per batch
NSC = S // P                # 16 s-chunks per batch

_compiled = None


def _build_program():
    import concourse.bacc as bacc
    import concourse.mybir as mybir
    import concourse.tile as tile

    f32 = mybir.dt.float32
    f32r = mybir.dt.float32r
    AF = mybir.ActivationFunctionType

    nc = bacc.Bacc("TRN2", target_bir_lowering=False, debug=False,
                   num_devices=NCORES)

    enc_t = nc.declare_dram_parameter("enc_t", [D, BLOC, S], f32r, isOutput=False)
    enc_n = nc.declare_dram_parameter("enc_n", [S, BLOC, D], f32r, isOutput=False)
    dec_t = nc.declare_dram_parameter("dec_t", [D, BLOC], f32r, isOutput=False)
    w_t = nc.declare_dram_parameter("w_t", [D, E], f32r, isOutput=False)
    sw_row = nc.declare_dram_parameter("sw_row", [1, E], f32, isOutput=False)
    out_d = nc.declare_dram_parameter("out", [BLOC, D], f32, isOutput=True)

    with tile.TileContext(nc) as tc:
        with tc.tile_pool(name="const", bufs=1) as const, \
             tc.tile_pool(name="et", bufs=3) as et_pool, \
             tc.tile_pool(name="h", bufs=3) as h_pool, \
             tc.tile_pool(name="en", bufs=4) as en_pool, \
             tc.tile_pool(name="misc", bufs=2) as misc, \
             tc.tile_pool(name="ph", bufs=4, space="PSUM") as ph_pool, \
             tc.tile_pool(name="pout", bufs=1, space="PSUM") as po_pool, \
             tc.tile_pool(name="psmall", bufs=1, space="PSUM") as psmall:

            enc_t_r = enc_t.ap().rearrange("(dc p) b s -> p dc b s", p=P)
            enc_n_r = enc_n.ap().rearrange("(sc p) b d -> p sc b d", p=P)
            w_t_r = w_t.ap().rearrange("(dc p) e -> p dc e", p=P)

            # ---- tiny gating DMAs first, then per-d-chunk splits ----
            dect_sb = const.tile([P, DC, BLOC], f32r)
            nc.sync.dma_start(dect_sb[:],
                              dec_t.ap().rearrange("(dc p) b -> p dc b", p=P))
            swr_sb = const.tile([1, E], f32)
            nc.sync.dma_start(swr_sb[:], sw_row.ap())
            et0 = et_pool.tile([P, DC, SB], f32r)
            w_sb = const.tile([P, DC, E], f32r)
            for dc in range(DC):
                nc.sync.dma_start(w_sb[:, dc, :], w_t_r[:, dc, :])
                nc.sync.dma_start(et0[:, dc, :], enc_t_r[:, dc, 0, 0:SB])
            ones_sb = const.tile([P, 1], f32)
            nc.vector.memset(ones_sb[:], 1.0)

            # ---- sw broadcast across partitions ----
            swbc_sb = const.tile([P, E], f32)
            nc.gpsimd.partition_broadcast(swbc_sb[:], swr_sb[:])

            # ---- decW: one M=4 chain, rows extracted + broadcast ----
            decw4 = const.tile([BLOC, E], f32)
            for g in range(E // SB):
                pdw = psmall.tile([BLOC, SB], f32, tag="pdw")
                for dc in range(DC):
                    nc.tensor.matmul(
                        pdw[:], dect_sb[:, dc, :], w_sb[:, dc, g * SB:(g + 1) * SB],
                        start=(dc == 0), stop=(dc == DC - 1))
                nc.scalar.copy(decw4[:, g * SB:(g + 1) * SB], pdw[:])
            decw_bc = const.tile([P, BLOC, E], f32)
            for b in range(BLOC):
                row = const.tile([1, E], f32, tag=f"dwrow{b}")
                nc.sync.dma_start(row[:], decw4[b:b + 1, :])
                nc.gpsimd.partition_broadcast(decw_bc[:, b, :], row[:])

            PASS2_DELAY = 4

            def _emit_pass2(b, sc, p_sb, po):
                # pass-2: po += p_col.T @ enc_chunk (unnormalized)
                en = en_pool.tile([P, D], f32r, tag="en", name=f"en{b}_{sc}")
                nc.sync.dma_start(en[:], enc_n_r[:, sc, b, :])
                for g in range(D // SB):
                    nc.tensor.matmul(
                        po[0:1, g * SB:(g + 1) * SB],
                        p_sb[:, sc:sc + 1], en[:, g * SB:(g + 1) * SB],
                        start=(sc == 0), stop=(sc == NSC - 1))

            for b in range(BLOC):
                scores = misc.tile([P, NSC], f32, tag="scores")
                p_sb = misc.tile([P, NSC], f32r, tag="p")
                po = po_pool.tile([1, D], f32)
                pending = []
                for sblk in range(NSBLK):
                    if b == 0 and sblk == 0:
                        et = et0
                    else:
                        et = et_pool.tile([P, DC, SB], f32r, tag="et0")
                        for dc in range(DC):
                            nc.sync.dma_start(
                                et[:, dc, :],
                                enc_t_r[:, dc, b, sblk * SB:(sblk + 1) * SB])
                    for j in range(SB // P):
                        sc = sblk * (SB // P) + j
                        h = h_pool.tile([P, E], f32)
                        phh = [ph_pool.tile([P, SB], f32, tag="phh",
                                            name=f"phh{g}")
                               for g in range(E // SB)]
                        for dc in range(DC):
                            for g in range(E // SB):
                                nc.tensor.matmul(
                                    phh[g][:], et[:, dc, j * P:(j + 1) * P],
                                    w_sb[:, dc, g * SB:(g + 1) * SB],
                                    start=(dc == 0), stop=(dc == DC - 1))
                        for g in range(E // SB):
                            # + decW[b, e] (varies along free dim -> DVE add)
                            nc.vector.tensor_add(
                                phh[g][:], phh[g][:],
                                decw_bc[:, b, g * SB:(g + 1) * SB])
                            nc.scalar.activation(
                                h[:, g * SB:(g + 1) * SB], phh[g][:], AF.Tanh)
                        # scores col: fused DVE (h * sw) with accum_out
                        g_scr = misc.tile([P, E], f32, tag="scratch")
                        nc.vector.scalar_tensor_tensor(
                            g_scr[:], h[:], 1.0, swbc_sb[:],
                            mybir.AluOpType.mult, mybir.AluOpType.mult,
                            accum_out=scores[:, sc:sc + 1])
                        nc.scalar.activation(p_sb[:, sc:sc + 1],
                                             scores[:, sc:sc + 1], AF.Exp)
                        pending.append(sc)
                        if len(pending) > PASS2_DELAY:
                            _emit_pass2(b, pending.pop(0), p_sb, po)

                for psc in pending:
                    _emit_pass2(b, psc, p_sb, po)

                # ---- softmax denominator and final evacuation ----
                acc = misc.tile([P, 1], f32, tag="acc")
                nc.vector.tensor_reduce(acc[:], p_sb[:], mybir.AxisListType.X,
                                        mybir.AluOpType.add)
                pl = psmall.tile([1, 1], f32, tag="pl")
                nc.tensor.matmul(pl[:], acc[:], ones_sb[:], start=True, stop=True)
                l_sb = misc.tile([1, 1], f32, tag="l")
                nc.scalar.copy(l_sb[:], pl[:])
                inv_l = misc.tile([1, 1], f32, tag="invl")
                nc.vector.reciprocal(inv_l[:], l_sb[:])
                out_sb = misc.tile([1, D], f32, tag="out")
                nc.scalar.activation(out_sb[:], po[:], AF.Copy, scale=inv_l[:])
                nc.sync.dma_start(out_d.ap()[b:b + 1, :], out_sb[:])

    nc.compile()
    return nc


def _get_program():
    global _compiled
    if _compiled is None:
        _compiled = _build_program()
    return _compiled


def make_in_maps(encoder_states, decoder_state, W_fc, score_w):
    """Shard + lay out full inputs into per-core input maps."""
    enc = np.asarray(encoder_states, dtype=np.float32)
    dec = np.asarray(decoder_state, dtype=np.float32)
    wfc = np.asarray(W_fc, dtype=np.float32)
    sw = np.asarray(score_w, dtype=np.float32)

    w_t = np.ascontiguousarray(wfc.T)                       # (D, E)
    sw_row = np.ascontiguousarray(sw[:, 0][None, :])        # (1, E)

    in_maps = []
    for i in range(NCORES):
        b0 = i * BLOC
        sl = enc[:, b0:b0 + BLOC, :]
        in_maps.append({
            "enc_t": np.ascontiguousarray(sl.transpose(2, 1, 0)),  # (D, BLOC, S)
            "enc_n": np.ascontiguousarray(sl),                     # (S, BLOC, D)
            "dec_t": np.ascontiguousarray(dec[b0:b0 + BLOC, :].T), # (D, BLOC)
            "w_t": w_t,
            "sw_row": sw_row,
        })
    return in_maps


def kernel(encoder_states, decoder_state, W_fc, score_w):
    from concourse.bass_utils import run_bass_kernel_spmd

    in_maps = make_in_maps(encoder_states, decoder_state, W_fc, score_w)
    nc = _get_program()
    res = run_bass_kernel_spmd(nc, in_maps, list(range(NCORES)))
    return np.concatenate([res.results[i]["out"] for i in range(NCORES)], axis=0)


# revision 25
# speedup vs baseline: 1.0219x; 1.0219x over previous
"""Trainium2 Bass kernel for the AttentionLoop module.

Reference computation (S=2048, B=32, D=1024, E=1024):
    h = tanh(einsum('sbd,ed->sbe', dec + enc, W_fc))
    scores = einsum('sbe,e->bs', h, score_w[:,0])
    attn = softmax(scores, axis=1)          # over seq
    out = einsum('bs,sbd->bd', attn, enc)   # (B, D)

Strategy: data-parallel over batch across 8 NeuronCores (4 batches/core),
core-local, no collectives.

Per-core kernel (v3). Heavy matmuls in float32r (1 cyc/row, full fp32 bits,
~1.5e-4 matmul rel err). The main matmul makes h (s, e)-oriented:
stationary = encT s-chunk (K=d, M=s), moving = W_T (K=d, N=e):
  - dc-outer / e-half-inner matmul order so each LDWEIGHTS (fp32r weight
    load is as long as one 512-col matmul) amortizes over two matmuls,
  - decoder bias decW[b, e] (varies along e) is a VectorE tensor_add on the
    PSUM tile against a pre-broadcast decw tile; decW is computed on-device
    by one M=4 matmul chain and replicated across partitions by GpSimd
    partition_broadcast,
  - scores[s] = sum_e h[s,e] sw[e] is one fused VectorE scalar_tensor_tensor
    (mult + accum_out) per s-chunk, landing directly as a column,
  - exp runs per column as soon as its scores are ready, so the pass-2
    weighted-sum matmuls (p column stationary, natural-enc moving)
    interleave with pass-1 and keep the TensorE dense.
W and encT DMAs are split per d-chunk so the first matmul starts ~3us in.
Softmax skips max-subtraction (scores are O(1); exp is safe in fp32).
l = sum(p) via DVE reduce + ones-matmul partition sum; 1/l is folded into
the final PSUM evacuation.
"""

import numpy as np

S, B, D, E = 2048, 32, 1024, 1024
NCORES = 8
BLOC = B // NCORES          # 4 batches per core
P = 128                     # partitions
DC = D // P                 # 8 d-chunks
SB = 512                    # moving free dim (PSUM bank)
NSBLK = S // SB             # 4 s-blocks per batch
NSC = S // P                # 16 s-chunks per batch

_compiled = None


def _build_program():
    import concourse.bacc as bacc
    import concourse.mybir as mybir
    import concourse.tile as tile

    f32 = mybir.dt.float32
    f32r = mybir.dt.float32r
    AF = mybir.ActivationFunctionType

    nc = bacc.Bacc("TRN2", target_bir_lowering=False, debug=False,
                   num_devices=NCORES)

    enc_t = nc.declare_dram_parameter("enc_t", [D, BLOC, S], f32r, isOutput=False)
    enc_n = nc.declare_dram_parameter("enc_n", [S, BLOC, D], f32r, isOutput=False)
    dec_t = nc.declare_dram_parameter("dec_t", [D, BLOC], f32r, isOutput=False)
    w_t = nc.declare_dram_parameter("w_t", [D, E], f32r, isOutput=False)
    sw_row = nc.declare_dram_parameter("sw_row", [1, E], f32, isOutput=False)
    out_d = nc.declare_dram_parameter("out", [BLOC, D], f32, isOutput=True)

    with tile.TileContext(nc) as tc:
        with tc.tile_pool(name="const", bufs=1) as const, \
             tc.tile_pool(name="et", bufs=3) as et_pool, \
             tc.tile_pool(name="h", bufs=3) as h_pool, \
             tc.tile_pool(name="en", bufs=4) as en_pool, \
             tc.tile_pool(name="misc", bufs=2) as misc, \
             tc.tile_pool(name="ph", bufs=4, space="PSUM") as ph_pool, \
             tc.tile_pool(name="pout", bufs=1, space="PSUM") as po_pool, \
             tc.tile_pool(name="psmall", bufs=1, space="PSUM") as psmall:

            enc_t_r = enc_t.ap().rearrange("(dc p) b s -> p dc b s", p=P)
            enc_n_r = enc_n.ap().rearrange("(sc p) b d -> p sc b d", p=P)
            w_t_r = w_t.ap().rearrange("(dc p) e -> p dc e", p=P)

            # ---- tiny gating DMAs first, then per-d-chunk splits ----
            dect_sb = const.tile([P, DC, BLOC], f32r)
            nc.sync.dma_start(dect_sb[:],
                              dec_t.ap().rearrange("(dc p) b -> p dc b", p=P))
            swr_sb = const.tile([1, E], f32)
            nc.sync.dma_start(swr_sb[:], sw_row.ap())
            et0 = et_pool.tile([P, DC, SB], f32r)
            w_sb = const.tile([P, DC, E], f32r)
            for dc in range(DC):
                nc.sync.dma_start(w_sb[:, dc, :], w_t_r[:, dc, :])
                nc.sync.dma_start(et0[:, dc, :], enc_t_r[:, dc, 0, 0:SB])
            ones_sb = const.tile([P, 1], f32)
            nc.vector.memset(ones_sb[:], 1.0)

            # ---- sw broadcast across partitions ----
            swbc_sb = const.tile([P, E], f32)
            nc.gpsimd.partition_broadcast(swbc_sb[:], swr_sb[:])

            # ---- decW: one M=4 chain, rows extracted + broadcast ----
            decw4 = const.tile([BLOC, E], f32)
            for g in range(E // SB):
                pdw = psmall.tile([BLOC, SB], f32, tag="pdw")
                for dc in range(DC):
                    nc.tensor.matmul(
                        pdw[:], dect_sb[:, dc, :], w_sb[:, dc, g * SB:(g + 1) * SB],
                        start=(dc == 0), stop=(dc == DC - 1))
                nc.scalar.copy(decw4[:, g * SB:(g + 1) * SB], pdw[:])
            decw_bc = const.tile([P, BLOC, E], f32)
            for b in range(BLOC):
                row = const.tile([1, E], f32, tag=f"dwrow{b}")
                nc.sync.dma_start(row[:], decw4[b:b + 1, :])
                nc.gpsimd.partition_broadcast(decw_bc[:, b, :], row[:])

            for b in range(BLOC):
                scores = misc.tile([P, NSC], f32, tag="scores")
                p_sb = misc.tile([P, NSC], f32r, tag="p")
                po = po_pool.tile([1, D], f32)
                for sblk in range(NSBLK):
                    if b == 0 and sblk == 0:
                        et = et0
                    else:
                        et = et_pool.tile([P, DC, SB], f32r, tag="et0")
                        for dc in range(DC):
                            nc.sync.dma_start(
                                et[:, dc, :],
                                enc_t_r[:, dc, b, sblk * SB:(sblk + 1) * SB])
                    for j in range(SB // P):
                        sc = sblk * (SB // P) + j
                        h = h_pool.tile([P, E], f32)
                        phh = [ph_pool.tile([P, SB], f32, tag="phh",
                                            name=f"phh{g}")
                               for g in range(E // SB)]
                        for dc in range(DC):
                            for g in range(E // SB):
                                nc.tensor.matmul(
                                    phh[g][:], et[:, dc, j * P:(j + 1) * P],
                                    w_sb[:, dc, g * SB:(g + 1) * SB],
                                    start=(dc == 0), stop=(dc == DC - 1))
                        for g in range(E // SB):
                            # + decW[b, e] (varies along free dim -> DVE add)
                            nc.vector.tensor_add(
                                phh[g][:], phh[g][:],
                                decw_bc[:, b, g * SB:(g + 1) * SB])
                            nc.scalar.activation(
                                h[:, g * SB:(g + 1) * SB], phh[g][:], AF.Tanh)
                        # scores col: fused DVE (h * sw) with accum_out
                        g_scr = misc.tile([P, E], f32, tag="scratch")
                        nc.vector.scalar_tensor_tensor(
                            g_scr[:], h[:], 1.0, swbc_sb[:],
                            mybir.AluOpType.mult, mybir.AluOpType.mult,
                            accum_out=scores[:, sc:sc + 1])
                        nc.scalar.activation(p_sb[:, sc:sc + 1],
                                             scores[:, sc:sc + 1], AF.Exp)
                        # pass-2: po += p_col.T @ enc_chunk (unnormalized)
                        en = en_pool.tile([P, D], f32r)
                        nc.sync.dma_start(en[:], enc_n_r[:, sc, b, :])
                        for g in range(D // SB):
                            nc.tensor.matmul(
                                po[0:1, g * SB:(g + 1) * SB],
                                p_sb[:, sc:sc + 1], en[:, g * SB:(g + 1) * SB],
                                start=(sc == 0), stop=(sc == NSC - 1))

                # ---- softmax denominator and final evacuation ----
                acc = misc.tile([P, 1], f32, tag="acc")
                nc.vector.tensor_reduce(acc[:], p_sb[:], mybir.AxisListType.X,
                                        mybir.AluOpType.add)
                pl = psmall.tile([1, 1], f32, tag="pl")
                nc.tensor.matmul(pl[:], acc[:], ones_sb[:], start=True, stop=True)
                l_sb = misc.tile([1, 1], f32, tag="l")
                nc.scalar.copy(l_sb[:], pl[:])
                inv_l = misc.tile([1, 1], f32, tag="invl")
                nc.vector.reciprocal(inv_l[:], l_sb[:])
                out_sb = misc.tile([1, D], f32, tag="out")
                nc.scalar.activation(out_sb[:], po[:], AF.Copy, scale=inv_l[:])
                nc.sync.dma_start(out_d.ap()[b:b + 1, :], out_sb[:])

    nc.compile()
    return nc


def _get_program():
    global _compiled
    if _compiled is None:
        _compiled = _build_program()
    return _compiled


def make_in_maps(encoder_states, decoder_state, W_fc, score_w):
    """Shard + lay out full inputs into per-core input maps."""
    enc = np.asarray(encoder_states, dtype=np.float32)
    dec = np.asarray(decoder_state, dtype=np.float32)
    wfc = np.asarray(W_fc, dtype=np.float32)
    sw = np.asarray(score_w, dtype=np.float32)

    w_t = np.ascontiguousarray(wfc.T)                       # (D, E)
    sw_row = np.ascontiguousarray(sw[:, 0][None, :])        # (1, E)

    in_maps = []
    for i in range(NCORES):
        b0 = i * BLOC
        sl = enc[:, b0:b0 + BLOC, :]
        in_maps.append({
            "enc_t": np.ascontiguousarray(sl.transpose(2, 1, 0)),  # (D, BLOC, S)
            "enc_n": np.ascontiguousarray(sl),                     # (S, BLOC, D)
            "dec_t": np.ascontiguousarray(dec[b0:b0 + BLOC, :].T), # (D, BLOC)
            "w_t": w_t,
            "sw_row": sw_row,
        })
    return in_maps


def kernel(encoder_states, decoder_state, W_fc, score_w):
    from concourse.bass_utils import run_bass_kernel_spmd

    in_maps = make_in_maps(encoder_states, decoder_state, W_fc, score_w)
    nc = _get_program()
    res = run_bass_kernel_spmd(nc, in_maps, list(range(NCORES)))
    return np.concatenate([res.results[i]["out"] for i in range(NCORES)], axis=0)
